# revision 1
# baseline (speedup 1.0000x reference)
"""Trainium2 Bass kernel for nn_MultiHeadAttention_81363860455568.

Reference computation (B=2, S=2048, D=1024, H=16, DK=64):
    qh = split_heads(q @ Wq.T); kh, vh likewise
    scores = softmax(qh @ kh.T / 8, axis=-1)
    scores = scores * reaches[:,None,None,:]            (per key)
    scores = scores * (1 - 0.999999*eye(S))             (diagonal suppression)
    out = vh - scores @ vh
    out = out * contrib[:,None,:,None]                  (per query)
    y = concat_heads(out) @ Wo.T

Sharding: 8 cores = 2 batches x 4 head-groups (4 heads each). Each core
receives its batch's transposed activations qT/kT/vT [D, S] in bf16 plus the
head-group slices of Wq/Wk/Wv (as [D, 256]) and Wo (as [256, D]), and returns
a partial y [S, D] (fp32) that the host sums across the 4 head-groups.

On-chip layout (per core, all matmuls bf16 with fp32 PSUM):
  - qhT/khT "pair" buffers [128, S]: heads (2p, 2p+1) stacked on partitions.
  - scoresT[k, q] via row-packed (K=64) matmul pairs; exp on ACT (scale=1/8).
  - diagonal handling: d2 = e*eye999 is subtracted from e in-place; the
    denominator matmul adds d2 back so softmax normalization sees unmasked e.
  - denominators: ones-vector matmul (col-packed M=1 tiles); AV: col-packed
    M=64 tiles with lhsT = reaches-scaled V in natural [k, d] layout.
  - epilogue: concatT = vhT - (AVT * 1/denom); contrib applied on the Wo
    output as a per-partition scalar.
"""

import functools

import numpy as np
import ml_dtypes

import concourse.bass as bass
import concourse.mybir as mybir
import concourse.tile as tile
from concourse import bacc
from concourse.bass_utils import run_bass_kernel_spmd
from concourse.masks import make_identity

BF16 = mybir.dt.bfloat16
F32 = mybir.dt.float32

B, S, D, H = 2, 2048, 1024, 16
DK = D // H          # 64
HG = 4               # heads per core (head group)
GD = HG * DK         # 256 head-group dims per core
NKC = D // 128       # 8 contraction chunks for projections
NKB = S // 128       # 16 key blocks
NMS = S // 128       # 16 query/row blocks
NQC = S // 512       # 4 query chunks of 512


def _emit_kernel(tc: tile.TileContext):
    nc = tc.nc

    qT = nc.declare_dram_parameter("qT", [D, S], BF16, isOutput=False).ap()
    kT = nc.declare_dram_parameter("kT", [D, S], BF16, isOutput=False).ap()
    vT = nc.declare_dram_parameter("vT", [D, S], BF16, isOutput=False).ap()
    wq = nc.declare_dram_parameter("wq", [D, GD], BF16, isOutput=False).ap()
    wk = nc.declare_dram_parameter("wk", [D, GD], BF16, isOutput=False).ap()
    wv = nc.declare_dram_parameter("wv", [D, GD], BF16, isOutput=False).ap()
    wo = nc.declare_dram_parameter("wo", [GD, D], BF16, isOutput=False).ap()
    rcol = nc.declare_dram_parameter("rcol", [128, NKB], F32, isOutput=False).ap()
    ccol = nc.declare_dram_parameter("ccol", [128, NMS], F32, isOutput=False).ap()
    y = nc.declare_dram_parameter("y", [S, D], F32, isOutput=True).ap()

    Exp = mybir.ActivationFunctionType.Exp

    # ---------------- resident SBUF buffers ----------------
    consts = tc.alloc_tile_pool(name="consts", bufs=1)
    wq_sb = consts.tile([128, NKC, GD], BF16)
    wk_sb = consts.tile([128, NKC, GD], BF16)
    wv_sb = consts.tile([128, NKC, GD], BF16)
    wo_sb = consts.tile([128, 2, D], BF16)
    rr = consts.tile([128, NKB], F32)
    cc = consts.tile([128, NMS], F32)
    eye999 = consts.tile([128, 128], F32)
    ident = consts.tile([128, 128], BF16)
    ones1 = consts.tile([128, 1], BF16)
    ones_row = consts.tile([1, 128], BF16)

    res = tc.alloc_tile_pool(name="res", bufs=1)
    qhT2 = [res.tile([128, S], BF16, name=f"qhT2_{p}") for p in range(2)]
    khT2 = [res.tile([128, S], BF16, name=f"khT2_{p}") for p in range(2)]
    vhT2 = [res.tile([128, S], BF16, name=f"vhT2_{p}") for p in range(2)]
    vnat = res.tile([128, NKB, GD], BF16)   # reaches-scaled V, natural [k, d]
    catT = [res.tile([128, S], BF16, name=f"catT_{p}") for p in range(2)]
    consts.seal()
    res.seal()

    # constant setup
    nc.gpsimd.memset(ones1, 1.0)
    nc.gpsimd.memset(ones_row, 1.0)
    make_identity(nc, ident)
    nc.gpsimd.memset(eye999, 0.0)
    nc.gpsimd.affine_select(
        out=eye999, in_=eye999,
        compare_op=mybir.AluOpType.not_equal,
        fill=0.999999, base=0, pattern=[[-1, 128]], channel_multiplier=1,
    )

    # constant/weight DMAs — V-projection operands first so PE starts early
    for kc in range(NKC):
        nc.sync.dma_start(out=wv_sb[:, kc, :], in_=wv[kc * 128:(kc + 1) * 128, :])
    nc.sync.dma_start(out=rr, in_=rcol)

    # ---------------- projection phase ----------------
    # spsum is allocated OUTSIDE the projection pools so the scores matmuls
    # of the attention phase get PSUM banks disjoint from the projection
    # banks and can start before the projection PSUM pipeline drains.
    spsum_cm = tc.tile_pool(name="spsum", bufs=2, space="PSUM")
    spsum = spsum_cm.__enter__()
    with (
        tc.tile_pool(name="xres", bufs=1) as xres,
        tc.tile_pool(name="ppsum", bufs=2, space="PSUM") as ppsum,
        tc.tile_pool(name="tpsum", bufs=2, space="PSUM") as tpsum,
        tc.tile_pool(name="vtmp_pool", bufs=2) as vtmp_pool,
    ):
        # Pre-load the exp activation table while ACT is otherwise idle so
        # the ~2.7us LoadActFuncSet is off the first real exp's critical path.
        wrm = vtmp_pool.tile([1, 2], F32, tag="wrm")
        nc.scalar.activation(wrm, ones_row[0:1, 0:2], Exp)

        vT_sb = xres.tile([128, NKC, S], BF16)
        qT_sb = xres.tile([128, NKC, S], BF16)
        kT_sb = xres.tile([128, NKC, S], BF16)
        for kc in range(NKC):
            nc.sync.dma_start(out=vT_sb[:, kc, :], in_=vT[kc * 128:(kc + 1) * 128, :])
        for kc in range(NKC):
            nc.sync.dma_start(out=wq_sb[:, kc, :], in_=wq[kc * 128:(kc + 1) * 128, :])
            nc.sync.dma_start(out=wk_sb[:, kc, :], in_=wk[kc * 128:(kc + 1) * 128, :])
        for kc in range(NKC):
            nc.sync.dma_start(out=qT_sb[:, kc, :], in_=qT[kc * 128:(kc + 1) * 128, :])
            nc.sync.dma_start(out=kT_sb[:, kc, :], in_=kT[kc * 128:(kc + 1) * 128, :])
        for p in range(2):
            nc.sync.dma_start(out=wo_sb[:, p, :], in_=wo[p * 128:(p + 1) * 128, :])
        nc.sync.dma_start(out=cc, in_=ccol)

        # V projection: vnat[k, d] (reaches-scaled) + vhT (unscaled, transposed)
        for ms in range(NMS):
            ps = ppsum.tile([128, 512], F32, tag="pp")
            for kc in range(NKC):
                nc.tensor.matmul(
                    ps[:, :GD],
                    lhsT=vT_sb[:, kc, ms * 128:(ms + 1) * 128],
                    rhs=wv_sb[:, kc, :],
                    start=(kc == 0), stop=(kc == NKC - 1),
                )
            nc.vector.tensor_scalar_mul(vnat[:, ms, :], ps[:, :GD], rr[:, ms:ms + 1])
            vtmp = vtmp_pool.tile([128, GD], BF16, tag="vtmp")
            nc.vector.tensor_copy(vtmp, ps[:, :GD])
            for p in range(2):
                tp = tpsum.tile([128, 128], BF16, tag="tp")
                nc.tensor.transpose(tp, vtmp[:, p * 128:(p + 1) * 128], ident)
                nc.vector.tensor_copy(vhT2[p][:, ms * 128:(ms + 1) * 128], tp)

        # Q/K projections into pair-stacked transposed layout
        for p in range(2):
            for (w_sb, dst) in ((wq_sb, qhT2), (wk_sb, khT2)):
                for nq in range(NQC):
                    ps = ppsum.tile([128, 512], F32, tag="pp")
                    for kc in range(NKC):
                        nc.tensor.matmul(
                            ps,
                            lhsT=w_sb[:, kc, p * 128:(p + 1) * 128],
                            rhs=(qT_sb if dst is qhT2 else kT_sb)[
                                :, kc, nq * 512:(nq + 1) * 512],
                            start=(kc == 0), stop=(kc == NKC - 1),
                        )
                    nc.vector.tensor_copy(dst[p][:, nq * 512:(nq + 1) * 512], ps)

    # ---------------- attention + output phase ----------------
    with (
        tc.tile_pool(name="apsum", bufs=1, space="PSUM") as apsum,
        tc.tile_pool(name="dwops", bufs=1, space="PSUM") as dwops,
        tc.tile_pool(name="epool", bufs=58) as epool,
        tc.tile_pool(name="d2pool", bufs=12) as d2pool,
        tc.tile_pool(name="mpool", bufs=4) as mpool,
        tc.tile_pool(name="ypool", bufs=3) as ypool,
    ):
        def emit_wo(mb, tail=False):
            for oc in range(2):
                wop = dwops.tile([128, 512], F32, tag="dwo", name="wop")
                for p in range(2):
                    nc.tensor.matmul(
                        wop,
                        lhsT=catT[p][:, mb * 128:(mb + 1) * 128],
                        rhs=wo_sb[:, p, oc * 512:(oc + 1) * 512],
                        start=(p == 0), stop=(p == 1),
                    )
                y_sb = ypool.tile([128, 512], F32, tag="ysb")
                if tail:
                    # ACT is idle at the kernel tail; DVE during the body
                    nc.scalar.activation(
                        y_sb, wop, mybir.ActivationFunctionType.Copy,
                        scale=cc[:, mb:mb + 1])
                else:
                    nc.vector.tensor_scalar_mul(y_sb, wop, cc[:, mb:mb + 1])
                nc.sync.dma_start(
                    out=y[mb * 128:(mb + 1) * 128, oc * 512:(oc + 1) * 512],
                    in_=y_sb,
                )

        def emit_b1(half, p):
            # ---- B1: scoresT -> exp -> diag -> denominator rows ----
            # The denominator matmuls ride along in B1 where PE is idle
            # under the ACT-bound exp stream.
            q0 = half * 1024
            etiles = {}   # (head_local, kb) -> [128, 1024] bf16
            dp = dwops.tile([128, 1024], F32, tag="dph")
            for kb in range(NKB):
                spair = []
                for h in range(2):
                    sp = spsum.tile([128, 1024], F32, tag="sc")
                    r0, r1 = h * 64, h * 64 + 64
                    for qc in range(2):
                        nc.tensor.matmul(
                            sp[:, qc * 512:(qc + 1) * 512],
                            lhsT=khT2[p][r0:r1, kb * 128:(kb + 1) * 128],
                            rhs=qhT2[p][r0:r1, q0 + qc * 512:q0 + (qc + 1) * 512],
                            start=True, stop=True,
                            tile_position=(h * 64, 0),
                        )
                    spair.append(sp)
                diag = 8 * half <= kb < 8 * half + 8
                off = 128 * (kb - 8 * half)
                d2s = {}
                for h in range(2):
                    et = epool.tile([128, 1024], BF16, tag="e")
                    nc.scalar.activation(et, spair[h], Exp, scale=0.125)
                    etiles[(h, kb)] = et
                    if diag:
                        d2 = d2pool.tile([128, 128], BF16, tag="d2")
                        nc.vector.tensor_mul(
                            d2, et[:, off:off + 128], eye999)
                        nc.vector.tensor_sub(
                            et[:, off:off + 128], et[:, off:off + 128], d2)
                        d2s[h] = d2
                # denominator rows (M=1 col-packed at col 0 / 32) over the
                # masked e; the diagonal contribution is added back from
                # d2 so normalization sees the unmasked sum. The add-back
                # must land after kb 0's start=True reset and before
                # kb 15's stop=True close.
                for h in range(2):
                    def addback():
                        nc.tensor.matmul(
                            dp[h * 32:h * 32 + 1, off:off + 128],
                            lhsT=ones1,
                            rhs=d2s[h],
                            start=False, stop=False,
                            tile_position=(0, h * 32),
                            skip_group_check=True,
                        )
                    if diag and kb > 0:
                        addback()
                    for qc in range(2):
                        nc.tensor.matmul(
                            dp[h * 32:h * 32 + 1, qc * 512:(qc + 1) * 512],
                            lhsT=ones1,
                            rhs=etiles[(h, kb)][:, qc * 512:(qc + 1) * 512],
                            start=(kb == 0), stop=(kb == NKB - 1),
                            tile_position=(0, h * 32),
                            skip_group_check=True,
                        )
                    if diag and kb == 0:
                        addback()
            return etiles, dp

        def emit_b2(half, p, etiles, dp, tail=False):
            # ---- B2: coefficients, then AV + epilogue per q chunk ----
            # Reciprocals/broadcast go first so the dph slot frees for the
            # next phase's B1 as early as possible.
            q0 = half * 1024
            bcCs = []
            for qc in range(2):
                wq0 = qc * 512
                bc = dwops.tile([128, 512], F32, tag="dwo", name="bc")
                for h in range(2):
                    c2 = mpool.tile([1, 512], F32, tag="c2", name=f"c2_{h}")
                    nc.vector.reciprocal(c2, dp[h * 32:h * 32 + 1, wq0:wq0 + 512])
                    # bf16 operands: fp32 matmuls lower to HI/LO passes on PE
                    c2b = mpool.tile([1, 512], BF16, tag="c2b", name=f"c2b_{h}")
                    nc.vector.tensor_copy(c2b, c2)
                    nc.tensor.matmul(
                        bc[h * 64:h * 64 + 64, :],
                        lhsT=ones_row[0:1, 0:64],
                        rhs=c2b,
                        start=True, stop=True,
                        tile_position=(0, h * 64),
                        skip_group_check=True,
                    )
                bcC = mpool.tile([128, 512], F32, tag="bc", name=f"bcC_{qc}")
                nc.vector.tensor_copy(bcC, bc)
                bcCs.append(bcC)
            for qc in range(2):
                wq0 = qc * 512
                av = apsum.tile([128, 512], F32, tag="av")
                for kb in range(NKB):
                    for h in range(2):
                        nc.tensor.matmul(
                            av[h * 64:h * 64 + 64, :],
                            lhsT=vnat[:, kb, p * 128 + h * 64:p * 128 + h * 64 + 64],
                            rhs=etiles[(h, kb)][:, wq0:wq0 + 512],
                            start=(kb == 0), stop=(kb == NKB - 1),
                            tile_position=(0, h * 64),
                            skip_group_check=True,
                        )
                # epilogue: catT = vhT - av * (1/denom)
                t1 = mpool.tile([128, 512], BF16, tag="t1")
                nc.vector.tensor_mul(t1, av, bcCs[qc])
                nc.vector.tensor_sub(
                    catT[p][:, q0 + wq0:q0 + wq0 + 512],
                    vhT2[p][:, q0 + wq0:q0 + wq0 + 512],
                    t1,
                )
                # Wo for this q chunk once both pairs' epilogues are done
                if p == 1:
                    for m in range(4):
                        emit_wo(8 * half + 4 * qc + m, tail=tail)

        # Software pipeline: emit B2 of phase i-1 after B1 of phase i so the
        # AV/epilogue PE work interleaves under the next phase's exp stream.
        steps = [(half, p) for half in range(2) for p in range(2)]
        pending = None
        for st in steps:
            made = emit_b1(*st)
            if pending is not None:
                emit_b2(*pending[0], *pending[1])
            pending = (st, made)
        emit_b2(*pending[0], *pending[1], tail=True)
    spsum_cm.__exit__(None, None, None)


@functools.cache
def build_nc() -> bass.Bass:
    nc = bacc.Bacc("TRN2", target_bir_lowering=False, debug=False)
    with tile.TileContext(nc) as tc:
        _emit_kernel(tc)
    nc.compile()
    return nc


def _prep_inputs(q, k, v, reaches, Wq, Wk, Wv, Wo):
    """Host-side shard + layout prep. Returns per-core input maps."""
    bf16 = ml_dtypes.bfloat16
    r = np.asarray(reaches, np.float32)
    rs = r.sum(axis=-1, keepdims=True)
    contrib = (rs - r) / (rs + 1e-9) * (1.0 - r) * 100.0  # [B, S] f32

    per_batch = []
    for b in range(B):
        qTb = np.ascontiguousarray(np.asarray(q[b], np.float32).T.astype(bf16))
        kTb = np.ascontiguousarray(np.asarray(k[b], np.float32).T.astype(bf16))
        vTb = np.ascontiguousarray(np.asarray(v[b], np.float32).T.astype(bf16))
        # [128, NKB] with [p, c] = vec[128*c + p]
        rcol = np.ascontiguousarray(r[b].reshape(NKB, 128).T)
        ccol = np.ascontiguousarray(contrib[b].reshape(NMS, 128).T)
        per_batch.append((qTb, kTb, vTb, rcol, ccol))

    in_maps = []
    for c in range(8):
        b, g = divmod(c, 4)
        hs = slice(g * GD, (g + 1) * GD)
        qTb, kTb, vTb, rcol, ccol = per_batch[b]
        in_maps.append({
            "qT": qTb, "kT": kTb, "vT": vTb,
            "wq": np.ascontiguousarray(np.asarray(Wq, np.float32)[hs, :].T).astype(bf16),
            "wk": np.ascontiguousarray(np.asarray(Wk, np.float32)[hs, :].T).astype(bf16),
            "wv": np.ascontiguousarray(np.asarray(Wv, np.float32)[hs, :].T).astype(bf16),
            "wo": np.ascontiguousarray(np.asarray(Wo, np.float32)[:, hs].T).astype(bf16),
            "rcol": rcol, "ccol": ccol,
        })
    return in_maps


def kernel(q, k, v, reaches, Wq, Wk, Wv, Wo, **run_kwargs):
    nc = build_nc()
    in_maps = _prep_inputs(q, k, v, reaches, Wq, Wk, Wv, Wo)
    res = run_bass_kernel_spmd(nc, in_maps, list(range(8)), **run_kwargs)
    out = np.zeros((B, S, D), np.float32)
    for c in range(8):
        b = c // 4
        out[b] += res.results[c]["y"]
    if run_kwargs:
        kernel.last_results = res
    return out



# revision 23
# speedup vs baseline: 1.6553x; 1.6553x over previous
"""Trainium2 Bass kernel for nn_MultiHeadAttention_81363860455568.

Reference computation (B=2, S=2048, D=1024, H=16, DK=64):
    qh = split_heads(q @ Wq.T); kh, vh likewise
    scores = softmax(qh @ kh.T / 8, axis=-1)
    scores = scores * reaches[:,None,None,:]            (per key)
    scores = scores * (1 - 0.999999*eye(S))             (diagonal suppression)
    out = vh - scores @ vh
    out = out * contrib[:,None,:,None]                  (per query)
    y = concat_heads(out) @ Wo.T

Sharding: 8 cores = 2 batches x 4 head-groups (4 heads each). Each core
receives its batch's transposed activations qT/kT (fp8e4m3) and vT (bf16)
[D, S] plus the head-group slices of Wq/Wk (fp8, pre-scaled x8), Wv (bf16,
as [D, 256]) and Wo (bf16, [256, D]), and returns a partial y [S, D] (fp32)
that the host sums across the 4 head-groups.

Cost-model-driven layout: matmul cost is output-columns x cycle regardless
of K, and fp8 DoubleRow halves it while contracting TWO K-planes, so every
long contraction runs as fp8 DR pairs:
  - Q/K projections: 4 DR matmuls over kc-pairs (inputs/weights fp8; W
    pre-scaled x8 so fp8 quantization stays in the normal range; the exp
    scale absorbs the 64x logit scale).
  - scores: qhT/khT stored [128, 2, S] fp8 with plane1 zeroed; DR with a
    zero second plane halves the per-column cost.
  - softmax denominators: ones-vector DR matmuls over kb-pairs into dp
    rows (M=1 col-packed), with the diagonal add-back pattern.
  - AV: DR over kb-pairs; lhsT = reaches-scaled V (vaug, fp8), rhs = exp
    scores (et, fp8 written directly by ACT with bias=-2 to keep values in
    fp8 range; softmax is shift-invariant).
V projection and Wo stay bf16: vh feeds the output directly (out = vh - ...)
so fp8 error there would exceed the accuracy budget. The per-query contrib
scale is folded into the V-projection transpose path (where q is on the
partition axis), removing all post-Wo scales.
"""

import functools

import numpy as np
import ml_dtypes

import concourse.bass as bass
import concourse.mybir as mybir
import concourse.tile as tile
from concourse import bacc
from concourse.bass_utils import run_bass_kernel_spmd
from concourse.masks import make_identity

BF16 = mybir.dt.bfloat16
F32 = mybir.dt.float32
F8 = mybir.dt.float8e4

B, S, D, H = 2, 2048, 1024, 16
DK = D // H          # 64
HG = 4               # heads per core (head group)
GD = HG * DK         # 256 head-group dims per core
NKC = D // 128       # 8 contraction chunks for projections
NKB = S // 128       # 16 key blocks
NMS = S // 128       # 16 query/row blocks
NQC = S // 512       # 4 query chunks of 512

DR = mybir.MatmulPerfMode.DoubleRow
EXP_SCALE = 0.125 / 64.0   # 1/sqrt(DK) / (8x8 weight prescale)
EXP_BIAS = -4.5            # shift-invariant; keeps exp below fp8e4m3's +-240
                           # (IEEE e4m3 with inf: overflow -> inf -> NaN);
                           # max logit in-distribution ~9.3 -> exp arg <= ~4.9


class TailPool:
    """Routes tile requests onto the tail PSUM pool, widening the wo ring
    by cycling the tag name (each tag gets its own slot in a bufs=1 pool)."""

    def __init__(self, pool):
        self.pool = pool
        self.n = 0

    def tile(self, shape, dtype, tag=None, name=None):
        if tag == "dwo":
            self.n += 1
            tag = f"dwo{self.n % 4}"
        return self.pool.tile(shape, dtype, tag=tag, name=name or tag)


def _emit_kernel(tc: tile.TileContext):
    nc = tc.nc

    # activations/weights come in pre-permuted to [128, chunk, cols] so a
    # single DMACopy instruction (one HWDGE occupancy) moves each slice
    qT = nc.declare_dram_parameter("qT", [128, NKC, S], F8, isOutput=False).ap()
    kT = nc.declare_dram_parameter("kT", [128, NKC, S], F8, isOutput=False).ap()
    vT = nc.declare_dram_parameter("vT", [128, NKC, S], BF16, isOutput=False).ap()
    wq = nc.declare_dram_parameter("wq", [128, NKC, GD], F8, isOutput=False).ap()
    wk = nc.declare_dram_parameter("wk", [128, NKC, GD], F8, isOutput=False).ap()
    wv = nc.declare_dram_parameter("wv", [128, NKC, GD], BF16, isOutput=False).ap()
    wo = nc.declare_dram_parameter("wo", [128, 2, D], BF16, isOutput=False).ap()
    rcol = nc.declare_dram_parameter("rcol", [128, NKB], F32, isOutput=False).ap()
    ccol = nc.declare_dram_parameter("ccol", [128, NMS], F32, isOutput=False).ap()
    crow = nc.declare_dram_parameter("crow", [1, S], BF16, isOutput=False).ap()
    y = nc.declare_dram_parameter("y", [S, D], BF16, isOutput=True).ap()

    Exp = mybir.ActivationFunctionType.Exp

    # ---------------- resident SBUF buffers ----------------
    consts = tc.alloc_tile_pool(name="consts", bufs=1)
    wq_sb = consts.tile([128, NKC, GD], F8)
    wk_sb = consts.tile([128, NKC, GD], F8)
    wv_sb = consts.tile([128, NKC, GD], BF16)
    wo_sb = consts.tile([128, 2, D], BF16)
    rr = consts.tile([128, NKB], F32)
    cc = consts.tile([128, NMS], F32)
    crow_sb = consts.tile([1, S], BF16)
    eye999 = consts.tile([128, 128], F32)
    ident = consts.tile([128, 128], BF16)
    ones1_8 = consts.tile([128, 1], F8)
    ones_row = consts.tile([1, 128], BF16)
    bias_m2 = consts.tile([128, 1], F32)

    res = tc.alloc_tile_pool(name="res", bufs=1)
    # q/k heads, transposed, fp8, DoubleRow layout: [h_local*64+d, plane, q]
    # with plane1 zeroed (DR sums both planes; the zero plane halves cost).
    q8 = [res.tile([128, 2, S], F8, name=f"q8_{p}") for p in range(2)]
    k8 = [res.tile([128, 2, S], F8, name=f"k8_{p}") for p in range(2)]
    vhT2 = [res.tile([128, S], BF16, name=f"vhT2_{p}") for p in range(2)]
    # reaches-scaled V in fp8, natural [k, d], 65 cols per head: 64 dims
    # + a ones column that makes the AV matmul also produce the softmax
    # denominator (row 64 of the [65, 512] psum tile)
    # (head blocks padded 65 -> 96 cols: dual-fp8 ldweights requires
    # 32-multiple plane strides/offsets)
    vaug = res.tile([128, NKB, 4, 96], F8)
    catT = [res.tile([128, S], BF16, name=f"catT_{p}") for p in range(2)]
    consts.seal()
    res.seal()

    # constant setup — pair-0 plane1 memsets first: they gate the first
    # scores matmul (DR reads both planes)
    nc.gpsimd.memset(k8[0][:, 1, :], 0.0)
    nc.gpsimd.memset(q8[0][:, 1, :], 0.0)
    nc.gpsimd.memset(k8[1][:, 1, :], 0.0)
    nc.gpsimd.memset(q8[1][:, 1, :], 0.0)
    nc.gpsimd.memset(ones1_8, 1.0)
    nc.gpsimd.memset(ones_row, 1.0)
    for _h in range(4):
        nc.gpsimd.memset(vaug[:, :, _h, 64:65], 1.0)

    nc.gpsimd.memset(bias_m2, EXP_BIAS)
    make_identity(nc, ident)
    nc.gpsimd.memset(eye999, 0.0)
    nc.gpsimd.affine_select(
        out=eye999, in_=eye999,
        compare_op=mybir.AluOpType.not_equal,
        fill=0.999999, base=0, pattern=[[-1, 128]], channel_multiplier=1,
    )
    # weight/constant DMAs — Q/K-projection operands first so PE starts early
    nc.sync.dma_start(out=wq_sb, in_=wq)
    nc.sync.dma_start(out=wk_sb, in_=wk)

    spsum_cm = tc.tile_pool(name="spsum", bufs=2, space="PSUM")
    spsum = spsum_cm.__enter__()

    # long-lived SBUF pools for the attention phase (entered before the
    # projection pools so pool exits stay LIFO; the first b1 is emitted
    # during the projection phase and needs epool/d2pool)
    epool_cm = tc.tile_pool(name="epool", bufs=4)
    epool = epool_cm.__enter__()
    d2pool_cm = tc.tile_pool(name="d2pool", bufs=33)
    d2pool = d2pool_cm.__enter__()
    mpool_cm = tc.tile_pool(name="mpool", bufs=2)
    mpool = mpool_cm.__enter__()
    ypool_cm = tc.tile_pool(name="ypool", bufs=5)
    ypool = ypool_cm.__enter__()
    apsum = None
    dwops = None

    xres_cm = tc.tile_pool(name="xres", bufs=1)
    xres = xres_cm.__enter__()
    ppsum_cm = tc.tile_pool(name="ppsum", bufs=2, space="PSUM")
    ppsum = ppsum_cm.__enter__()
    tpsum_cm = tc.tile_pool(name="tpsum", bufs=2, space="PSUM")
    tpsum = tpsum_cm.__enter__()
    vtmp_cm = tc.tile_pool(name="vtmp_pool", bufs=2)
    vtmp_pool = vtmp_cm.__enter__()

    qT_sb = xres.tile([128, NKC, S], F8)
    kT_sb = xres.tile([128, NKC, S], F8)
    vT_sb = xres.tile([128, NKC, S], BF16)
    for nq in range(NQC):
        for (x_sb, xd) in ((kT_sb, kT), (qT_sb, qT)):
            nc.sync.dma_start(
                out=x_sb[:, :, nq * 512:(nq + 1) * 512],
                in_=xd[:, :, nq * 512:(nq + 1) * 512])
    nc.sync.dma_start(out=rr, in_=rcol)
    nc.sync.dma_start(out=wv_sb, in_=wv)
    nc.sync.dma_start(out=vT_sb, in_=vT)
    nc.sync.dma_start(out=wo_sb, in_=wo)
    nc.sync.dma_start(out=cc, in_=ccol)
    nc.sync.dma_start(out=crow_sb, in_=crow)

    # Pre-load the exp activation table while ACT is otherwise idle so
    # the ~2.7us LoadActFuncSet is off the first real exp's critical path.
    wrm = vtmp_pool.tile([1, 2], F32, tag="wrm")
    nc.scalar.activation(wrm, ones_row[0:1, 0:2], Exp)

    def emit_qk_proj(p):
        # Q/K projections into DR-plane-0 of the pair-stacked fp8 layout,
        # nq-major and k-first so the first scores block unblocks early.
        for nq in range(NQC):
            for (w_sb, x_sb, dst) in ((wk_sb, kT_sb, k8), (wq_sb, qT_sb, q8)):
                ps = ppsum.tile([128, 512], F32, tag="pp")
                for kc2 in range(NKC // 2):
                    nc.tensor.matmul(
                        ps,
                        lhsT=w_sb[:, 2 * kc2:2 * kc2 + 2, p * 128:(p + 1) * 128],
                        rhs=x_sb[:, 2 * kc2:2 * kc2 + 2, nq * 512:(nq + 1) * 512],
                        start=(kc2 == 0), stop=(kc2 == NKC // 2 - 1),
                        perf_mode=DR,
                    )
                nc.vector.tensor_copy(dst[p][:, 0, nq * 512:(nq + 1) * 512], ps)

    def emit_v_proj():
        # V projection: vaug[k, d] (reaches-scaled, fp8 for AV DR) and
        # vhT (contrib-scaled, transposed, bf16 for the epilogue).
        for ms in range(NMS):
            ps = ppsum.tile([128, 512], F32, tag="pp")
            for kc in range(NKC):
                nc.tensor.matmul(
                    ps[:, :GD],
                    lhsT=vT_sb[:, kc, ms * 128:(ms + 1) * 128],
                    rhs=wv_sb[:, kc, :],
                    start=(kc == 0), stop=(kc == NKC - 1),
                )
            nc.vector.tensor_scalar_mul(
                vaug[:, ms, :, 0:64], ps[:, :GD], rr[:, ms:ms + 1])
            vtmp = vtmp_pool.tile([128, GD], BF16, tag="vtmp")
            nc.vector.tensor_scalar_mul(vtmp, ps[:, :GD], cc[:, ms:ms + 1])
            for p in range(2):
                tp = tpsum.tile([128, 128], BF16, tag="tp")
                nc.tensor.transpose(tp, vtmp[:, p * 128:(p + 1) * 128], ident)
                nc.vector.tensor_copy(vhT2[p][:, ms * 128:(ms + 1) * 128], tp)

    # ---------------- attention + output phase ----------------

    def emit_wo(mb, tail=False):
        for oc in range(2):
            wop = dwops.tile([128, 512], F32, tag="dwo", name="wop")
            for p in range(2):
                nc.tensor.matmul(
                    wop,
                    lhsT=catT[p][:, mb * 128:(mb + 1) * 128],
                    rhs=wo_sb[:, p, oc * 512:(oc + 1) * 512],
                    start=(p == 0), stop=(p == 1),
                )
            y_sb = ypool.tile([128, 512], BF16, tag="ysb")
            if tail and oc == 1:
                # ACT is idle at the kernel tail; split copies ACT/DVE
                nc.scalar.activation(
                    y_sb, wop, mybir.ActivationFunctionType.Copy)
            else:
                nc.vector.tensor_copy(y_sb, wop)
            nc.sync.dma_start(
                out=y[mb * 128:(mb + 1) * 128, oc * 512:(oc + 1) * 512],
                in_=y_sb,
            )

    def emit_b1(half, p, mask_on_pool=False):
        # ---- B1: scoresT (fp8 DR) -> exp (fp8 out) -> diag masking ----
        # The denominator matmuls ride along (PE is idle under the
        # ACT-bound exp stream) except for the first b1, which is emitted
        # during the projection phase before the dwops PSUM pool opens.
        q0 = half * 1024
        ets = [epool.tile([128, NKB, 1024], F8, tag="et", name=f"et{h}")
               for h in range(2)]
        d2s = {}
        for kb in range(NKB):
            spair = []
            for h in range(2):
                sp = spsum.tile([128, 1024], F32, tag="sc")
                r0, r1 = h * 64, h * 64 + 64
                for qc in range(2):
                    nc.tensor.matmul(
                        sp[:, qc * 512:(qc + 1) * 512],
                        lhsT=k8[p][r0:r1, :, kb * 128:(kb + 1) * 128],
                        rhs=q8[p][r0:r1, :, q0 + qc * 512:q0 + (qc + 1) * 512],
                        start=True, stop=True,
                        perf_mode=DR,
                    )
                spair.append(sp)
            diag = 8 * half <= kb < 8 * half + 8
            off = 128 * (kb - 8 * half)
            for h in range(2):
                nc.scalar.activation(
                    ets[h][:, kb, :], spair[h], Exp,
                    scale=EXP_SCALE, bias=bias_m2[:, 0:1])
                if diag:
                    # In fp8, 0.999999*e rounds back to e, so the masked
                    # diagonal is exactly zero and the add-back restores
                    # the full diagonal term for the denominator.
                    d2 = d2pool.tile([128, 128], F8, tag="d2")
                    eng = nc.gpsimd if mask_on_pool else nc.vector
                    eng.tensor_mul(
                        d2, ets[h][:, kb, off:off + 128], eye999)
                    eng.tensor_sub(
                        ets[h][:, kb, off:off + 128],
                        ets[h][:, kb, off:off + 128], d2)
                    d2s[(h, kb)] = d2
        return ets, d2s

    def emit_b2(half, p, ets, d2s, tail=False):
        # ---- B2: AV+denominator (fp8 DR, both heads at dst partition 0 —
        # the hardware requires DR results to start at partition 0), then
        # per-head softmax coefficients and epilogue, Wo per q chunk ----
        q0 = half * 1024
        for qc in range(2):
            wq0 = qc * 512
            avs = []
            for h in range(2):
                av = apsum.tile([65, 512], F32, tag=f"avh{h}", name=f"av{h}")
                for j in range(NKB // 2):
                    if j == NKB // 2 - 1:
                        # diagonal add-back into the denominator row, inside
                        # the accumulation group (before the stop)
                        for dkb in range(8 * half + 4 * qc,
                                         8 * half + 4 * qc + 4):
                            nc.tensor.matmul(
                                av[64:65,
                                   128 * (dkb - 8 * half) - wq0:
                                   128 * (dkb - 8 * half) - wq0 + 128],
                                lhsT=ones1_8,
                                rhs=d2s[(h, dkb)],
                                start=False, stop=False,
                                tile_position=(0, 64),
                                skip_group_check=True,
                            )
                    nc.tensor.matmul(
                        av,
                        lhsT=vaug[:, 2 * j:2 * j + 2, p * 2 + h, 0:65],
                        rhs=ets[h][:, 2 * j:2 * j + 2, wq0:wq0 + 512],
                        start=(j == 0), stop=(j == NKB // 2 - 1),
                        skip_group_check=True,
                        perf_mode=DR,
                    )
                avs.append(av)
            t1 = mpool.tile([128, 512], BF16, tag="t1")
            for h in range(2):
                c2 = mpool.tile([1, 512], F32, tag="c2", name=f"c2_{h}")
                nc.vector.reciprocal(c2, avs[h][64:65, :])
                c2b = mpool.tile([1, 512], BF16, tag="c2b", name=f"c2b_{h}")
                (nc.gpsimd if tail else nc.vector).tensor_mul(
                    c2b, c2, crow_sb[0:1, q0 + wq0:q0 + wq0 + 512])
                bch = dwops.tile([64, 512], F32, tag="bch", name="bch")
                nc.tensor.matmul(
                    bch,
                    lhsT=ones_row[0:1, 0:64],
                    rhs=c2b,
                    start=True, stop=True,
                    skip_group_check=True,
                )
                bcC = mpool.tile([64, 512], F32, tag="bc", name="bcC")
                if tail:
                    nc.scalar.activation(
                        bcC, bch, mybir.ActivationFunctionType.Copy)
                else:
                    nc.vector.tensor_copy(bcC, bch)
                if h == 0:
                    nc.vector.tensor_mul(t1[0:64, :], avs[h][0:64, :], bcC)
                else:
                    # h1 is computed at base partition 0; a PE identity
                    # matmul moves it to partitions 64-127 (engines cannot
                    # move data across partitions)
                    t1b = mpool.tile([64, 512], BF16, tag="t1b", name="t1b")
                    nc.vector.tensor_mul(t1b, avs[h][0:64, :], bcC)
                    mv = dwops.tile([128, 512], F32, tag="dwo", name="mv")
                    nc.tensor.matmul(
                        mv[64:128, :],
                        lhsT=ident[0:64, 0:64],
                        rhs=t1b,
                        start=True, stop=True,
                        tile_position=(0, 64),
                        skip_group_check=True,
                    )
                    nc.vector.tensor_copy(t1[64:128, :], mv[64:128, :])
            (nc.gpsimd if tail else nc.vector).tensor_sub(
                catT[p][:, q0 + wq0:q0 + wq0 + 512],
                vhT2[p][:, q0 + wq0:q0 + wq0 + 512],
                t1,
            )
            # Wo for this q chunk once both pairs' epilogues are done
            if p == 1:
                for m in range(4):
                    emit_wo(8 * half + 4 * qc + m, tail=tail)

    # Software pipeline: first b1 rides right after the pair-0 Q/K
    # projection so ACT starts early; V projection and pair-1 Q/K
    # projection fill PE under the first exp stream. Then emit B2 of
    # phase i-1 after B1 of phase i so the AV/epilogue PE work
    # interleaves under the next phase's exp stream.
    emit_qk_proj(0)
    made0 = emit_b1(0, 0)
    emit_qk_proj(1)
    emit_v_proj()

    vtmp_cm.__exit__(None, None, None)
    tpsum_cm.__exit__(None, None, None)
    ppsum_cm.__exit__(None, None, None)
    xres_cm.__exit__(None, None, None)

    apsum_cm = tc.tile_pool(name="apsum", bufs=1, space="PSUM")
    apsum = apsum_cm.__enter__()
    dwops_cm = tc.tile_pool(name="dwops", bufs=1, space="PSUM")
    dwops = dwops_cm.__enter__()

    steps = [(0, 1), (1, 0), (1, 1)]
    pending = ((0, 0), made0)
    for st in steps:
        made = emit_b1(*st, mask_on_pool=(st == (1, 1)))
        emit_b2(*pending[0], *pending[1])
        pending = (st, made)

    # Tail: no exp stream left to hide under, so the last b2 is a pure
    # latency chain. Swap the PSUM pools (release is dependency-tracked,
    # not a barrier) for a multi-buffered wo ring so the 16 Wo tiles
    # pipeline matmul/copy/DMA instead of serializing on one bank.
    dwops_cm.__exit__(None, None, None)
    apsum_cm.__exit__(None, None, None)
    spsum_cm.__exit__(None, None, None)
    tail_cm = tc.tile_pool(name="tailp", bufs=1, space="PSUM")
    tailp = tail_cm.__enter__()
    apsum = dwops = TailPool(tailp)
    emit_b2(*pending[0], *pending[1], tail=True)
    tail_cm.__exit__(None, None, None)

    for cm in (ypool_cm, mpool_cm, d2pool_cm, epool_cm):
        cm.__exit__(None, None, None)


@functools.cache
def build_nc() -> bass.Bass:
    nc = bacc.Bacc("TRN2", target_bir_lowering=False, debug=False)
    with tile.TileContext(nc) as tc:
        _emit_kernel(tc)
    nc.compile()
    return nc


def _prep_inputs(q, k, v, reaches, Wq, Wk, Wv, Wo):
    """Host-side shard + layout prep. Returns per-core input maps."""
    bf16 = ml_dtypes.bfloat16
    f8 = ml_dtypes.float8_e4m3fn
    r = np.asarray(reaches, np.float32)
    rs = r.sum(axis=-1, keepdims=True)
    contrib = (rs - r) / (rs + 1e-9) * (1.0 - r) * 100.0  # [B, S] f32

    def chunked(xT, dt):
        # [D, S] -> [128, NKC, S] with (p, kc, c) = xT[kc*128 + p, c]
        return np.ascontiguousarray(
            xT.reshape(NKC, 128, -1).transpose(1, 0, 2)).astype(dt)

    per_batch = []
    for b in range(B):
        qTb = chunked(np.asarray(q[b], np.float32).T, f8)
        kTb = chunked(np.asarray(k[b], np.float32).T, f8)
        vTb = chunked(np.asarray(v[b], np.float32).T, bf16)
        # [128, NKB] with [p, c] = vec[128*c + p]
        rcol = np.ascontiguousarray(r[b].reshape(NKB, 128).T)
        ccol = np.ascontiguousarray(contrib[b].reshape(NMS, 128).T)
        crow_b = np.ascontiguousarray(contrib[b].reshape(1, S)).astype(bf16)
        per_batch.append((qTb, kTb, vTb, rcol, ccol, crow_b))

    in_maps = []
    for c in range(8):
        b, g = divmod(c, 4)
        hs = slice(g * GD, (g + 1) * GD)
        qTb, kTb, vTb, rcol, ccol, crow_b = per_batch[b]
        in_maps.append({
            "qT": qTb, "kT": kTb, "vT": vTb,
            "wq": chunked(np.asarray(Wq, np.float32)[hs, :].T * 8.0, f8),
            "wk": chunked(np.asarray(Wk, np.float32)[hs, :].T * 8.0, f8),
            "wv": chunked(np.asarray(Wv, np.float32)[hs, :].T, bf16),
            "wo": np.ascontiguousarray(
                np.asarray(Wo, np.float32)[:, hs].T.reshape(
                    2, 128, D).transpose(1, 0, 2)).astype(bf16),
            "rcol": rcol, "ccol": ccol, "crow": crow_b,
        })
    return in_maps


def kernel(q, k, v, reaches, Wq, Wk, Wv, Wo, **run_kwargs):
    nc = build_nc()
    in_maps = _prep_inputs(q, k, v, reaches, Wq, Wk, Wv, Wo)
    res = run_bass_kernel_spmd(nc, in_maps, list(range(8)), **run_kwargs)
    out = np.zeros((B, S, D), np.float32)
    for c in range(8):
        b = c // 4
        out[b] += np.asarray(res.results[c]["y"], np.float32)
    if run_kwargs:
        kernel.last_results = res
    return out


# revision 33
# speedup vs baseline: 1.6600x; 1.0028x over previous
"""Trainium2 Bass kernel for nn_MultiHeadAttention_81363860455568.

Reference computation (B=2, S=2048, D=1024, H=16, DK=64):
    qh = split_heads(q @ Wq.T); kh, vh likewise
    scores = softmax(qh @ kh.T / 8, axis=-1)
    scores = scores * reaches[:,None,None,:]            (per key)
    scores = scores * (1 - 0.999999*eye(S))             (diagonal suppression)
    out = vh - scores @ vh
    out = out * contrib[:,None,:,None]                  (per query)
    y = concat_heads(out) @ Wo.T

Sharding: 8 cores = 2 batches x 4 head-groups (4 heads each). Each core
receives its batch's transposed activations qT/kT (fp8e4m3) and vT (bf16)
[D, S] plus the head-group slices of Wq/Wk (fp8, pre-scaled x8), Wv (bf16,
as [D, 256]) and Wo (bf16, [256, D]), and returns a partial y [S, D] (fp32)
that the host sums across the 4 head-groups.

Cost-model-driven layout: matmul cost is output-columns x cycle regardless
of K, and fp8 DoubleRow halves it while contracting TWO K-planes, so every
long contraction runs as fp8 DR pairs:
  - Q/K projections: 4 DR matmuls over kc-pairs (inputs/weights fp8; W
    pre-scaled x8 so fp8 quantization stays in the normal range; the exp
    scale absorbs the 64x logit scale).
  - scores: qhT/khT stored [128, 2, S] fp8 with plane1 zeroed; DR with a
    zero second plane halves the per-column cost.
  - softmax denominators: ones-vector DR matmuls over kb-pairs into dp
    rows (M=1 col-packed), with the diagonal add-back pattern.
  - AV: DR over kb-pairs; lhsT = reaches-scaled V (vaug, fp8), rhs = exp
    scores (et, fp8 written directly by ACT with bias=-2 to keep values in
    fp8 range; softmax is shift-invariant).
V projection and Wo stay bf16: vh feeds the output directly (out = vh - ...)
so fp8 error there would exceed the accuracy budget. The per-query contrib
scale is folded into the V-projection transpose path (where q is on the
partition axis), removing all post-Wo scales.
"""

import functools

import numpy as np
import ml_dtypes

import concourse.bass as bass
import concourse.mybir as mybir
import concourse.tile as tile
from concourse import bacc
from concourse.bass_utils import run_bass_kernel_spmd
from concourse.masks import make_identity

BF16 = mybir.dt.bfloat16
F32 = mybir.dt.float32
F8 = mybir.dt.float8e4

B, S, D, H = 2, 2048, 1024, 16
DK = D // H          # 64
HG = 4               # heads per core (head group)
GD = HG * DK         # 256 head-group dims per core
NKC = D // 128       # 8 contraction chunks for projections
NKB = S // 128       # 16 key blocks
NMS = S // 128       # 16 query/row blocks
NQC = S // 512       # 4 query chunks of 512

DR = mybir.MatmulPerfMode.DoubleRow
# Schraudolph exp-as-bit-trick constants for the DVE-offloaded tiles:
# exp(sp/512 - 4.5) ~= bitcast_f32(int32(sp * SCH_A + SCH_B))
_LOG2E = 1.4426950408889634
SCH_A = _LOG2E / 512.0 * 8388608.0
SCH_B = (127.0 - 4.5 * _LOG2E) * 8388608.0 - 366393.0
SCH_KBS = (5, 13)   # per-phase kb tiles whose exp runs on DVE instead of ACT
EXP_SCALE = 0.125 / 64.0   # 1/sqrt(DK) / (8x8 weight prescale)
EXP_BIAS = -4.5            # shift-invariant; keeps exp below fp8e4m3's +-240
                           # (IEEE e4m3 with inf: overflow -> inf -> NaN);
                           # max logit in-distribution ~9.3 -> exp arg <= ~4.9


class TailPool:
    """Routes tile requests onto the tail PSUM pool, widening the wo ring
    by cycling the tag name (each tag gets its own slot in a bufs=1 pool)."""

    def __init__(self, pool):
        self.pool = pool
        self.n = 0

    def tile(self, shape, dtype, tag=None, name=None):
        if tag == "dwo":
            self.n += 1
            tag = f"dwo{self.n % 4}"
        return self.pool.tile(shape, dtype, tag=tag, name=name or tag)


def _emit_kernel(tc: tile.TileContext):
    nc = tc.nc

    # activations/weights come in pre-permuted to [128, chunk, cols] so a
    # single DMACopy instruction (one HWDGE occupancy) moves each slice
    qT = nc.declare_dram_parameter("qT", [128, NKC, S], F8, isOutput=False).ap()
    kT = nc.declare_dram_parameter("kT", [128, NKC, S], F8, isOutput=False).ap()
    vT = nc.declare_dram_parameter("vT", [128, NKC, S], BF16, isOutput=False).ap()
    wq = nc.declare_dram_parameter("wq", [128, NKC * GD], F8, isOutput=False).ap()
    wk = nc.declare_dram_parameter("wk", [128, NKC * GD], F8, isOutput=False).ap()
    wv = nc.declare_dram_parameter("wv", [128, NKC, GD], BF16, isOutput=False).ap()
    wo = nc.declare_dram_parameter("wo", [128, 2, D], BF16, isOutput=False).ap()
    rcol = nc.declare_dram_parameter("rcol", [128, NKB], F32, isOutput=False).ap()
    ccol = nc.declare_dram_parameter("ccol", [128, NMS], F32, isOutput=False).ap()
    crow = nc.declare_dram_parameter("crow", [1, S], BF16, isOutput=False).ap()
    y = nc.declare_dram_parameter("y", [S, D], BF16, isOutput=True).ap()

    Exp = mybir.ActivationFunctionType.Exp

    # ---------------- resident SBUF buffers ----------------
    consts = tc.alloc_tile_pool(name="consts", bufs=1)
    wq_sb = consts.tile([128, NKC, GD], F8)
    wk_sb = consts.tile([128, NKC, GD], F8)
    wv_sb = consts.tile([128, NKC, GD], BF16)
    wo_sb = consts.tile([128, 2, D], BF16)
    rr = consts.tile([128, NKB], F32)
    cc = consts.tile([128, NMS], F32)
    crow_sb = consts.tile([1, S], BF16)
    eye999 = consts.tile([128, 128], F32)
    ident = consts.tile([128, 128], BF16)
    ones1_8 = consts.tile([128, 1], F8)
    ones_row = consts.tile([1, 128], BF16)
    bias_m2 = consts.tile([128, 1], F32)

    res = tc.alloc_tile_pool(name="res", bufs=1)
    # q/k heads, transposed, fp8, DoubleRow layout: [h_local*64+d, plane, q]
    # with plane1 zeroed (DR sums both planes; the zero plane halves cost).
    q8 = [res.tile([128, 2, S], F8, name=f"q8_{p}") for p in range(2)]
    k8 = [res.tile([128, 2, S], F8, name=f"k8_{p}") for p in range(2)]
    vhT2 = [res.tile([128, S], BF16, name=f"vhT2_{p}") for p in range(2)]
    # reaches-scaled V in fp8, natural [k, d], 65 cols per head: 64 dims
    # + a ones column that makes the AV matmul also produce the softmax
    # denominator (row 64 of the [65, 512] psum tile)
    # (head blocks padded 65 -> 96 cols: dual-fp8 ldweights requires
    # 32-multiple plane strides/offsets)
    vaug = res.tile([128, NKB, 4, 96], F8)
    catT = [res.tile([128, S], BF16, name=f"catT_{p}") for p in range(2)]
    consts.seal()
    res.seal()

    # constant setup — pair-0 plane1 memsets first: they gate the first
    # scores matmul (DR reads both planes)
    nc.gpsimd.memset(k8[0][:, 1, :], 0.0)
    nc.gpsimd.memset(q8[0][:, 1, :], 0.0)
    nc.gpsimd.memset(k8[1][:, 1, :], 0.0)
    nc.gpsimd.memset(q8[1][:, 1, :], 0.0)
    nc.gpsimd.memset(ones1_8, 1.0)
    nc.gpsimd.memset(ones_row, 1.0)
    for _h in range(4):
        nc.gpsimd.memset(vaug[:, :, _h, 64:65], 1.0)

    nc.gpsimd.memset(bias_m2, EXP_BIAS)
    make_identity(nc, ident)
    nc.gpsimd.memset(eye999, 0.0)
    nc.gpsimd.affine_select(
        out=eye999, in_=eye999,
        compare_op=mybir.AluOpType.not_equal,
        fill=0.999999, base=0, pattern=[[-1, 128]], channel_multiplier=1,
    )
    # weight/constant DMAs — Q/K-projection operands first so PE starts early
    nc.sync.dma_start(out=wq_sb, in_=wq)
    nc.sync.dma_start(out=wk_sb, in_=wk)

    spsum_cm = tc.tile_pool(name="spsum", bufs=2, space="PSUM")
    spsum = spsum_cm.__enter__()

    # long-lived SBUF pools for the attention phase (entered before the
    # projection pools so pool exits stay LIFO; the first b1 is emitted
    # during the projection phase and needs epool/d2pool)
    epool_cm = tc.tile_pool(name="epool", bufs=4)
    epool = epool_cm.__enter__()
    d2pool_cm = tc.tile_pool(name="d2pool", bufs=33)
    d2pool = d2pool_cm.__enter__()
    mpool_cm = tc.tile_pool(name="mpool", bufs=2)
    mpool = mpool_cm.__enter__()
    ypool_cm = tc.tile_pool(name="ypool", bufs=5)
    ypool = ypool_cm.__enter__()
    apsum = None
    dwops = None

    xres_cm = tc.tile_pool(name="xres", bufs=1)
    xres = xres_cm.__enter__()
    ppsum_cm = tc.tile_pool(name="ppsum", bufs=2, space="PSUM")
    ppsum = ppsum_cm.__enter__()
    tpsum_cm = tc.tile_pool(name="tpsum", bufs=2, space="PSUM")
    tpsum = tpsum_cm.__enter__()
    vtmp_cm = tc.tile_pool(name="vtmp_pool", bufs=2)
    vtmp_pool = vtmp_cm.__enter__()

    qT_sb = xres.tile([128, NKC, S], F8)
    kT_sb = xres.tile([128, NKC, S], F8)
    vT_sb = xres.tile([128, NKC, S], BF16)
    for nq in range(NQC):
        for (x_sb, xd) in ((kT_sb, kT), (qT_sb, qT)):
            nc.sync.dma_start(
                out=x_sb[:, :, nq * 512:(nq + 1) * 512],
                in_=xd[:, :, nq * 512:(nq + 1) * 512])
    nc.sync.dma_start(out=rr, in_=rcol)
    nc.sync.dma_start(out=wv_sb, in_=wv)
    nc.sync.dma_start(out=vT_sb, in_=vT)
    nc.sync.dma_start(out=wo_sb, in_=wo)
    nc.sync.dma_start(out=cc, in_=ccol)
    nc.sync.dma_start(out=crow_sb, in_=crow)

    # Pre-load the exp activation table while ACT is otherwise idle so
    # the ~2.7us LoadActFuncSet is off the first real exp's critical path.
    wrm = vtmp_pool.tile([1, 2], F32, tag="wrm")
    nc.scalar.activation(wrm, ones_row[0:1, 0:2], Exp)

    def emit_qk_proj(p):
        # Q/K projections into DR-plane-0 of the pair-stacked fp8 layout,
        # nq-major and k-first so the first scores block unblocks early.
        for nq in range(NQC):
            for (w_sb, x_sb, dst) in ((wk_sb, kT_sb, k8), (wq_sb, qT_sb, q8)):
                ps = ppsum.tile([128, 512], F32, tag="pp")
                for kc2 in range(NKC // 2):
                    nc.tensor.matmul(
                        ps,
                        lhsT=w_sb[:, 2 * kc2:2 * kc2 + 2, p * 128:(p + 1) * 128],
                        rhs=x_sb[:, 2 * kc2:2 * kc2 + 2, nq * 512:(nq + 1) * 512],
                        start=(kc2 == 0), stop=(kc2 == NKC // 2 - 1),
                        perf_mode=DR,
                    )
                nc.vector.tensor_copy(dst[p][:, 0, nq * 512:(nq + 1) * 512], ps)

    def emit_v_proj():
        # V projection: vaug[k, d] (reaches-scaled, fp8 for AV DR) and
        # vhT (contrib-scaled, transposed, bf16 for the epilogue).
        for ms in range(NMS):
            ps = ppsum.tile([128, 512], F32, tag="pp")
            for kc in range(NKC):
                nc.tensor.matmul(
                    ps[:, :GD],
                    lhsT=vT_sb[:, kc, ms * 128:(ms + 1) * 128],
                    rhs=wv_sb[:, kc, :],
                    start=(kc == 0), stop=(kc == NKC - 1),
                )
            nc.vector.tensor_scalar_mul(
                vaug[:, ms, :, 0:64], ps[:, :GD], rr[:, ms:ms + 1])
            vtmp = vtmp_pool.tile([128, GD], BF16, tag="vtmp")
            nc.vector.tensor_scalar_mul(vtmp, ps[:, :GD], cc[:, ms:ms + 1])
            for p in range(2):
                tp = tpsum.tile([128, 128], BF16, tag="tp")
                nc.tensor.transpose(tp, vtmp[:, p * 128:(p + 1) * 128], ident)
                nc.vector.tensor_copy(vhT2[p][:, ms * 128:(ms + 1) * 128], tp)

    # ---------------- attention + output phase ----------------

    def emit_wo(mb, tail=False):
        for oc in range(2):
            wop = dwops.tile([128, 512], F32, tag="dwo", name="wop")
            for p in range(2):
                nc.tensor.matmul(
                    wop,
                    lhsT=catT[p][:, mb * 128:(mb + 1) * 128],
                    rhs=wo_sb[:, p, oc * 512:(oc + 1) * 512],
                    start=(p == 0), stop=(p == 1),
                )
            y_sb = ypool.tile([128, 512], BF16, tag="ysb")
            if tail and oc == 1:
                # ACT is idle at the kernel tail; split copies ACT/DVE
                nc.scalar.activation(
                    y_sb, wop, mybir.ActivationFunctionType.Copy)
            else:
                nc.vector.tensor_copy(y_sb, wop)
            nc.sync.dma_start(
                out=y[mb * 128:(mb + 1) * 128, oc * 512:(oc + 1) * 512],
                in_=y_sb,
            )

    def emit_b1(half, p, mask_on_pool=False):
        # ---- B1: scoresT (fp8 DR) -> exp (fp8 out) -> diag masking ----
        # The denominator matmuls ride along (PE is idle under the
        # ACT-bound exp stream) except for the first b1, which is emitted
        # during the projection phase before the dwops PSUM pool opens.
        q0 = half * 1024
        ets = [epool.tile([128, NKB, 1024], F8, tag="et", name=f"et{h}")
               for h in range(2)]
        d2s = {}
        for kb in range(NKB):
            spair = []
            for h in range(2):
                sp = spsum.tile([128, 1024], F32, tag="sc")
                r0, r1 = h * 64, h * 64 + 64
                for qc in range(2):
                    nc.tensor.matmul(
                        sp[:, qc * 512:(qc + 1) * 512],
                        lhsT=k8[p][r0:r1, :, kb * 128:(kb + 1) * 128],
                        rhs=q8[p][r0:r1, :, q0 + qc * 512:q0 + (qc + 1) * 512],
                        start=True, stop=True,
                        perf_mode=DR,
                    )
                spair.append(sp)
            diag = 8 * half <= kb < 8 * half + 8
            off = 128 * (kb - 8 * half)
            for h in range(2):
                nc.scalar.activation(
                    ets[h][:, kb, :], spair[h], Exp,
                    scale=EXP_SCALE, bias=bias_m2[:, 0:1])
                if diag:
                    # In fp8, 0.999999*e rounds back to e, so the masked
                    # diagonal is exactly zero and the add-back restores
                    # the full diagonal term for the denominator.
                    d2 = d2pool.tile([128, 128], F8, tag="d2")
                    eng = nc.gpsimd
                    eng.tensor_mul(
                        d2, ets[h][:, kb, off:off + 128], eye999)
                    eng.tensor_sub(
                        ets[h][:, kb, off:off + 128],
                        ets[h][:, kb, off:off + 128], d2)
                    d2s[(h, kb)] = d2
        return ets, d2s

    def emit_b2(half, p, ets, d2s, tail=False):
        # ---- B2: AV+denominator (fp8 DR, both heads at dst partition 0 —
        # the hardware requires DR results to start at partition 0), then
        # per-head softmax coefficients and epilogue, Wo per q chunk ----
        q0 = half * 1024
        for qc in range(2):
            wq0 = qc * 512
            avs = []
            for h in range(2):
                av = apsum.tile([65, 512], F32, tag=f"avh{h}", name=f"av{h}")
                for j in range(NKB // 2):
                    if j == NKB // 2 - 1:
                        # diagonal add-back into the denominator row, inside
                        # the accumulation group (before the stop)
                        for dkb in range(8 * half + 4 * qc,
                                         8 * half + 4 * qc + 4):
                            nc.tensor.matmul(
                                av[64:65,
                                   128 * (dkb - 8 * half) - wq0:
                                   128 * (dkb - 8 * half) - wq0 + 128],
                                lhsT=ones1_8,
                                rhs=d2s[(h, dkb)],
                                start=False, stop=False,
                                tile_position=(0, 64),
                                skip_group_check=True,
                            )
                    nc.tensor.matmul(
                        av,
                        lhsT=vaug[:, 2 * j:2 * j + 2, p * 2 + h, 0:65],
                        rhs=ets[h][:, 2 * j:2 * j + 2, wq0:wq0 + 512],
                        start=(j == 0), stop=(j == NKB // 2 - 1),
                        skip_group_check=True,
                        perf_mode=DR,
                    )
                avs.append(av)
            t1 = mpool.tile([128, 512], BF16, tag="t1")
            for h in range(2):
                c2 = mpool.tile([1, 512], F32, tag="c2", name=f"c2_{h}")
                nc.vector.reciprocal(c2, avs[h][64:65, :])
                c2b = mpool.tile([1, 512], BF16, tag="c2b", name=f"c2b_{h}")
                nc.vector.tensor_mul(
                    c2b, c2, crow_sb[0:1, q0 + wq0:q0 + wq0 + 512])
                bch = dwops.tile([64, 512], F32, tag="bch", name="bch")
                nc.tensor.matmul(
                    bch,
                    lhsT=ones_row[0:1, 0:64],
                    rhs=c2b,
                    start=True, stop=True,
                    skip_group_check=True,
                )
                bcC = mpool.tile([64, 512], F32, tag="bc", name="bcC")
                if tail:
                    nc.scalar.activation(
                        bcC, bch, mybir.ActivationFunctionType.Copy)
                else:
                    nc.vector.tensor_copy(bcC, bch)
                if h == 0:
                    nc.vector.tensor_mul(t1[0:64, :], avs[h][0:64, :], bcC)
                else:
                    # h1 is computed at base partition 0; a PE identity
                    # matmul moves it to partitions 64-127 (engines cannot
                    # move data across partitions)
                    t1b = mpool.tile([64, 512], BF16, tag="t1b", name="t1b")
                    nc.vector.tensor_mul(t1b, avs[h][0:64, :], bcC)
                    mv = dwops.tile([128, 512], F32, tag="dwo", name="mv")
                    nc.tensor.matmul(
                        mv[64:128, :],
                        lhsT=ident[0:64, 0:64],
                        rhs=t1b,
                        start=True, stop=True,
                        tile_position=(0, 64),
                        skip_group_check=True,
                    )
                    nc.vector.tensor_copy(t1[64:128, :], mv[64:128, :])
            nc.vector.tensor_sub(
                catT[p][:, q0 + wq0:q0 + wq0 + 512],
                vhT2[p][:, q0 + wq0:q0 + wq0 + 512],
                t1,
            )
            # Wo for this q chunk once both pairs' epilogues are done
            if p == 1:
                for m in range(4):
                    emit_wo(8 * half + 4 * qc + m, tail=tail)

    # Software pipeline: first b1 rides right after the pair-0 Q/K
    # projection so ACT starts early; V projection and pair-1 Q/K
    # projection fill PE under the first exp stream. Then emit B2 of
    # phase i-1 after B1 of phase i so the AV/epilogue PE work
    # interleaves under the next phase's exp stream.
    emit_qk_proj(0)
    made0 = emit_b1(0, 0)
    emit_qk_proj(1)
    emit_v_proj()

    vtmp_cm.__exit__(None, None, None)
    tpsum_cm.__exit__(None, None, None)
    ppsum_cm.__exit__(None, None, None)
    xres_cm.__exit__(None, None, None)

    apsum_cm = tc.tile_pool(name="apsum", bufs=1, space="PSUM")
    apsum = apsum_cm.__enter__()
    dwops_cm = tc.tile_pool(name="dwops", bufs=1, space="PSUM")
    dwops = dwops_cm.__enter__()

    steps = [(0, 1), (1, 0), (1, 1)]
    pending = ((0, 0), made0)
    for st in steps:
        made = emit_b1(*st, mask_on_pool=(st == (1, 1)))
        emit_b2(*pending[0], *pending[1])
        pending = (st, made)

    # Tail: no exp stream left to hide under, so the last b2 is a pure
    # latency chain. Swap the PSUM pools (release is dependency-tracked,
    # not a barrier) for a multi-buffered wo ring so the 16 Wo tiles
    # pipeline matmul/copy/DMA instead of serializing on one bank.
    dwops_cm.__exit__(None, None, None)
    apsum_cm.__exit__(None, None, None)
    spsum_cm.__exit__(None, None, None)
    tail_cm = tc.tile_pool(name="tailp", bufs=1, space="PSUM")
    tailp = tail_cm.__enter__()
    apsum = dwops = TailPool(tailp)
    emit_b2(*pending[0], *pending[1], tail=True)
    tail_cm.__exit__(None, None, None)

    for cm in (ypool_cm, mpool_cm, d2pool_cm, epool_cm):
        cm.__exit__(None, None, None)


@functools.cache
def build_nc() -> bass.Bass:
    nc = bacc.Bacc("TRN2", target_bir_lowering=False, debug=False)
    with tile.TileContext(nc) as tc:
        _emit_kernel(tc)
    nc.compile()
    return nc


def _prep_inputs(q, k, v, reaches, Wq, Wk, Wv, Wo):
    """Host-side shard + layout prep. Returns per-core input maps."""
    bf16 = ml_dtypes.bfloat16
    f8 = ml_dtypes.float8_e4m3fn
    r = np.asarray(reaches, np.float32)
    rs = r.sum(axis=-1, keepdims=True)
    contrib = (rs - r) / (rs + 1e-9) * (1.0 - r) * 100.0  # [B, S] f32

    def chunked(xT, dt):
        # [D, S] -> [128, NKC, S] with (p, kc, c) = xT[kc*128 + p, c]
        return np.ascontiguousarray(
            xT.reshape(NKC, 128, -1).transpose(1, 0, 2)).astype(dt)

    per_batch = []
    for b in range(B):
        qTb = chunked(np.asarray(q[b], np.float32).T, f8)
        kTb = chunked(np.asarray(k[b], np.float32).T, f8)
        vTb = chunked(np.asarray(v[b], np.float32).T, bf16)
        # [128, NKB] with [p, c] = vec[128*c + p]
        rcol = np.ascontiguousarray(r[b].reshape(NKB, 128).T)
        ccol = np.ascontiguousarray(contrib[b].reshape(NMS, 128).T)
        crow_b = np.ascontiguousarray(contrib[b].reshape(1, S)).astype(bf16)
        per_batch.append((qTb, kTb, vTb, rcol, ccol, crow_b))

    in_maps = []
    for c in range(8):
        b, g = divmod(c, 4)
        hs = slice(g * GD, (g + 1) * GD)
        qTb, kTb, vTb, rcol, ccol, crow_b = per_batch[b]
        in_maps.append({
            "qT": qTb, "kT": kTb, "vT": vTb,
            "wq": chunked(np.asarray(Wq, np.float32)[hs, :].T * 8.0,
                          f8).reshape(128, NKC * GD),
            "wk": chunked(np.asarray(Wk, np.float32)[hs, :].T * 8.0,
                          f8).reshape(128, NKC * GD),
            "wv": chunked(np.asarray(Wv, np.float32)[hs, :].T, bf16),
            "wo": np.ascontiguousarray(
                np.asarray(Wo, np.float32)[:, hs].T.reshape(
                    2, 128, D).transpose(1, 0, 2)).astype(bf16),
            "rcol": rcol, "ccol": ccol, "crow": crow_b,
        })
    return in_maps


def kernel(q, k, v, reaches, Wq, Wk, Wv, Wo, **run_kwargs):
    nc = build_nc()
    in_maps = _prep_inputs(q, k, v, reaches, Wq, Wk, Wv, Wo)
    res = run_bass_kernel_spmd(nc, in_maps, list(range(8)), **run_kwargs)
    out = np.zeros((B, S, D), np.float32)
    for c in range(8):
        b = c // 4
        out[b] += np.asarray(res.results[c]["y"], np.float32)
    if run_kwargs:
        kernel.last_results = res
    return out
